# revision 14
# baseline (speedup 1.0000x reference)
"""NT-Xent (SimCLR) contrastive loss on 8 Trainium2 NeuronCores — v2.

Math: z = concat(z_i, z_j) [2B, D], zn = z / ||z||_row,
logits = zn @ zn.T / T (diag masked), targets pair row r with r±B.
loss = mean_r(LSE_r - 2*p_r),  LSE_r = log(S_r - diag_r) + 2,
  S_r = sum_c exp(2*s_rc - 2),  p_r = zn_r . zn_partner(r).

Sharding: data-parallel rows. Core k gets z pre-rotated by 1024k rows
(host np.roll), computes its 1024 rows x all 8192 cols.

v2 pipeline (per core):
  Host feeds bf16 natural z AND a bf16 transposed copy whose columns are
  permuted per-1024-chunk as i = 8q + t (q=0..127, t=0..7, row=128t+q).
  That makes the device-computed inv vector land LINEARLY in DRAM so it
  can be broadcast-read back across partitions with unit-stride DMA.
  Per chunk c: ss (DVE square+reduce) -> inv16 = exp(-ln(ss)/2 + ln16)
  (ACT, shares the ln/exp table set with the main loop) -> DRAM write +
  partition-replicated read -> normalize transposed chunk to fp8 e4m3
  (x16 scale) -> fp8 DoubleRow matmuls (full K=256 per instr, 0.5
  cyc/col) -> ACT exp(psum*2/256 - 2) with accum_out row sums.
  Pair logits from bf16 raw dots (rows 0..1023 x 4096..5119); host
  divides by fp64 norms. Diagonal approximated as exp(0)=1 at combine
  (error ~1e-4 of the row sum).
Host combine = the all-reduce: assemble S, subtract diag, log, mean.
"""

import math

import numpy as np
import ml_dtypes

import concourse.bacc as bacc
import concourse.mybir as mybir
import concourse.tile as tile
from concourse.bass_utils import run_bass_kernel_spmd

P = 128
D = 256
B = 4096
N2 = 2 * B            # 8192 rows
NCORES = 8
NCH = 8               # 1024-row/col chunks
CW = 1024             # chunk width
TPC = 8               # [128, D] row-tiles per natural chunk
G = 4                 # psum column groups of 2048
M_TILES = 8
TEMP = 0.5
CSTAB = 2.0
SCALE_EXP = 2.0 / 256.0   # psum holds 256*s
LN16 = math.log(16.0)

f32 = mybir.dt.float32
bf16 = mybir.dt.bfloat16
fp8 = mybir.dt.float8e4
AF = mybir.ActivationFunctionType
OP = mybir.AluOpType
DR = mybir.MatmulPerfMode.DoubleRow


def _emit(tc, znat_d, zt_d, s_out, rawp_out):
    nc = tc.nc

    with tc.tile_pool(name="nat", bufs=NCH) as nat_pool, \
            tc.tile_pool(name="zth", bufs=2 * NCH) as zth_pool, \
            tc.tile_pool(name="zn8", bufs=NCH) as zn8_pool, \
            tc.tile_pool(name="invb", bufs=NCH) as invb_pool, \
            tc.tile_pool(name="sqt", bufs=2) as sq_pool, \
            tc.tile_pool(name="ett", bufs=2) as et_pool, \
            tc.tile_pool(name="small", bufs=4 * NCH + 8) as small_pool, \
            tc.tile_pool(name="acc", bufs=4) as acc_pool, \
            tc.tile_pool(name="dram", bufs=2, space="DRAM") as dram_pool, \
            tc.tile_pool(name="psum", bufs=2, space="PSUM") as psum_pool:

        # Pre-load the combined ln+exp activation table set (id 6:
        # natural_log_exp_and_others) so per-chunk Ln and the main-loop
        # Exp never thrash tables (the auto pass alternates sets 5/0).
        nc.scalar.add_instruction(
            mybir.InstLoadActFuncSet(name="preload_lnexp",
                                     act_func_set_id=6, ins=[], outs=[]))
        bias_t = acc_pool.tile([P, 1], f32, tag="bias", name="bias_t")
        nc.vector.memset(bias_t[:], -CSTAB)
        ln16_t = acc_pool.tile([P, 1], f32, tag="ln16", name="ln16_t")
        nc.vector.memset(ln16_t[:], LN16)
        rs = acc_pool.tile([P, M_TILES * G], f32, tag="rs", name="rs")
        s_sb = acc_pool.tile([P, M_TILES], f32, tag="ssb", name="s_sb")
        rawp = acc_pool.tile([P, TPC], f32, tag="rawp", name="rawp")

        inv_d = dram_pool.tile([N2], bf16, tag="invd", name="inv_d")

        nats, zths, zn8s, invbs = [], [], [], []
        for c in range(NCH):
            # natural chunk: host pre-arranged to SBUF layout [128, 2048]
            nat = nat_pool.tile([P, TPC * D], bf16, tag="nat", name=f"nat{c}")
            nc.sync.dma_start(out=nat[:],
                              in_=znat_d[:, 2048 * c:2048 * (c + 1)])
            nats.append(nat)
            # transposed chunk halves: [128 d, 1024 cols] each
            hh = []
            for h in range(2):
                zth = zth_pool.tile([P, CW], fp8, tag="zth",
                                    name=f"zth{c}_{h}")
                nc.gpsimd.dma_start(out=zth[:],
                                    in_=zt_d[P * h:P * (h + 1),
                                             CW * c:CW * (c + 1)])
                hh.append(zth)
            zths.append(hh)

        # per-chunk pipeline: ss -> inv16 -> DRAM -> invb -> normalize,
        # so the main loop can start after chunks 0,1 (~10us).
        for c in range(NCH):
            sq = sq_pool.tile([P, TPC * D], bf16, tag="sq", name="sq")
            nc.vector.tensor_tensor(sq[:], nats[c][:], nats[c][:], op=OP.mult)
            ss_c = small_pool.tile([P, TPC], f32, tag="ss", name=f"ss{c}")
            nc.vector.reduce_sum(
                out=ss_c[:].unsqueeze(-1),
                in_=sq[:].rearrange("p (t d) -> p t d", t=TPC),
                axis=mybir.AxisListType.X)
            # inv16 = exp(-0.5 ln ss + ln 16)   (single preloaded table set)
            ln_c = small_pool.tile([P, TPC], f32, tag="ln", name=f"ln{c}")
            nc.scalar.activation(ln_c[:], ss_c[:], AF.Ln)
            iv_c = small_pool.tile([P, TPC], bf16, tag="iv", name=f"iv{c}")
            nc.scalar.activation(iv_c[:], ln_c[:], AF.Exp,
                                 scale=-0.5, bias=ln16_t[:])
            # DRAM roundtrip: linear write (addr = 1024c + 8p + t)
            nc.scalar.dma_start(
                out=inv_d[CW * c:CW * (c + 1)].rearrange("(p t) -> p t",
                                                         t=TPC),
                in_=iv_c[:])
            invb = invb_pool.tile([P, CW], bf16, tag="invb", name=f"invb{c}")
            nc.sync.dma_start(
                out=invb[:],
                in_=inv_d[CW * c:CW * (c + 1)].unsqueeze(0)
                    .broadcast_to([P, CW]))
            invbs.append(invb)
            # normalize transposed chunk -> fp8 (x16 via inv16)
            zn8 = zn8_pool.tile([P, 2 * CW], fp8, tag="zn8", name=f"zn8{c}")
            for h in range(2):
                nc.vector.tensor_tensor(
                    zn8[:, CW * h:CW * (h + 1)], zths[c][h][:], invb[:],
                    op=OP.mult)
            zn8s.append(zn8)

        # raw pair dots: rolled rows 0..1023 (chunk 0) x 4096..5119 (chunk 4)
        tt = sq_pool.tile([P, TPC * D], bf16, tag="sq", name="tt")
        nc.vector.tensor_tensor(tt[:], nats[0][:], nats[4][:], op=OP.mult)
        nc.vector.reduce_sum(
            out=rawp[:].unsqueeze(-1),
            in_=tt[:].rearrange("p (t d) -> p t d", t=TPC),
            axis=mybir.AxisListType.X)
        nc.sync.dma_start(out=rawp_out, in_=rawp[:])

        # main loop: 1024 own rows x 8192 cols, fp8 DoubleRow (K=256/instr)
        def lhsT(m):
            # own rows: chunk-0 cols i = 8q + m  ->  [128, 2, 128]
            return zn8s[0][:].rearrange("p (h q t) -> p h t q", h=2,
                                        t=TPC)[:, :, m, :]

        def rhs(chunk, off):
            return zn8s[chunk][:].rearrange(
                "p (h w) -> p h w", h=2)[:, :, off:off + 512]

        for g in range(G):
            for m in range(M_TILES):
                ps = psum_pool.tile([P, 2048], f32, tag="ps",
                                    name=f"ps{g}_{m}")
                for c4 in range(4):
                    nc.tensor.matmul(
                        out=ps[:, 512 * c4:512 * (c4 + 1)],
                        lhsT=lhsT(m),
                        rhs=rhs(2 * g + c4 // 2, 512 * (c4 % 2)),
                        start=True, stop=True, perf_mode=DR)
                et = et_pool.tile([P, 2048], bf16, tag="et", name=f"et{g}_{m}")
                idx = 4 * m + g
                nc.scalar.activation(et[:], ps[:], AF.Exp, bias=bias_t[:],
                                     scale=SCALE_EXP,
                                     accum_out=rs[:, idx:idx + 1])

        nc.vector.reduce_sum(
            out=s_sb[:].unsqueeze(-1),
            in_=rs[:].rearrange("p (m g) -> p m g", g=G),
            axis=mybir.AxisListType.X)
        nc.sync.dma_start(out=s_out, in_=s_sb[:])


def build():
    nc = bacc.Bacc("TRN2", target_bir_lowering=False, debug=False)
    znat = nc.dram_tensor("znat", [P, N2 * D // P], bf16,
                          kind="ExternalInput").ap()
    zt = nc.dram_tensor("zt", [D, N2], fp8, kind="ExternalInput").ap()
    s_out = nc.dram_tensor("s_out", [P, M_TILES], f32,
                           kind="ExternalOutput").ap()
    rawp_out = nc.dram_tensor("rawp_out", [P, TPC], f32,
                              kind="ExternalOutput").ap()
    with tile.TileContext(nc) as tc:
        _emit(tc, znat, zt, s_out, rawp_out)
    nc.compile()
    return nc


_COLMAP = None


def _colmap():
    global _COLMAP
    if _COLMAP is None:
        i = np.arange(N2)
        _COLMAP = (i // CW) * CW + (i % TPC) * P + (i % CW) // TPC
    return _COLMAP


def make_in_maps(z_i, z_j):
    z_full = np.concatenate(
        [np.asarray(z_i, dtype=np.float32), np.asarray(z_j, dtype=np.float32)],
        axis=0)
    rmap = _colmap()
    maps = []
    for k in range(NCORES):
        zr = np.roll(z_full, -CW * k, axis=0)
        # SBUF layout: znat[p, (c n d)] = zr[1024c + 128n + p, d]
        znat = np.ascontiguousarray(
            zr.reshape(NCH, TPC, P, D).transpose(2, 0, 1, 3)
            .reshape(P, NCH * TPC * D)).astype(ml_dtypes.bfloat16)
        zt = np.ascontiguousarray(zr.T[:, rmap]).astype(ml_dtypes.float8_e4m3fn)
        maps.append({"znat": znat, "zt": zt})
    return maps


def combine(results, z_full):
    n = np.linalg.norm(z_full.astype(np.float64), axis=1)
    S = np.empty(N2, np.float64)
    pv = np.empty(N2, np.float64)
    pp = np.arange(P)[:, None]
    mm = np.arange(M_TILES)[None, :]
    for k in range(NCORES):
        gidx = ((CW * k + P * mm + pp) % N2).ravel()
        S[gidx] = results[k]["s_out"].astype(np.float64).ravel()
        pv[gidx] = results[k]["rawp_out"].astype(np.float64).ravel()
    St = S - 1.0                       # drop diag (exp(2*d-2) ~= 1)
    lse = np.log(St) + CSTAB
    partner = (np.arange(N2) + B) % N2
    p = pv / (n * n[partner])
    loss = np.mean(lse - 2.0 * p)
    return np.asarray(loss, dtype=np.float32)


_NC_CACHE = None


def kernel(z_i, z_j):
    global _NC_CACHE
    if _NC_CACHE is None:
        _NC_CACHE = build()
    z_full = np.concatenate(
        [np.asarray(z_i, dtype=np.float32), np.asarray(z_j, dtype=np.float32)],
        axis=0)
    res = run_bass_kernel_spmd(
        _NC_CACHE, make_in_maps(z_i, z_j), list(range(NCORES))).results
    return combine(res, z_full)


# revision 16
# speedup vs baseline: 1.0553x; 1.0553x over previous
"""NT-Xent (SimCLR) contrastive loss on 8 Trainium2 NeuronCores — v2.

Math: z = concat(z_i, z_j) [2B, D], zn = z / ||z||_row,
logits = zn @ zn.T / T (diag masked), targets pair row r with r±B.
loss = mean_r(LSE_r - 2*p_r),  LSE_r = log(S_r - diag_r) + 2,
  S_r = sum_c exp(2*s_rc - 2),  p_r = zn_r . zn_partner(r).

Sharding: data-parallel rows. Core k gets z pre-rotated by 1024k rows
(host np.roll), computes its 1024 rows x all 8192 cols.

v2 pipeline (per core):
  Host feeds bf16 natural z AND a bf16 transposed copy whose columns are
  permuted per-1024-chunk as i = 8q + t (q=0..127, t=0..7, row=128t+q).
  That makes the device-computed inv vector land LINEARLY in DRAM so it
  can be broadcast-read back across partitions with unit-stride DMA.
  Per chunk c: ss (DVE square+reduce) -> inv16 = exp(-ln(ss)/2 + ln16)
  (ACT, shares the ln/exp table set with the main loop) -> DRAM write +
  partition-replicated read -> normalize transposed chunk to fp8 e4m3
  (x16 scale) -> fp8 DoubleRow matmuls (full K=256 per instr, 0.5
  cyc/col) -> ACT exp(psum*2/256 - 2) with accum_out row sums.
  Pair logits from bf16 raw dots (rows 0..1023 x 4096..5119); host
  divides by fp64 norms. Diagonal approximated as exp(0)=1 at combine
  (error ~1e-4 of the row sum).
Host combine = the all-reduce: assemble S, subtract diag, log, mean.
"""

import math

import numpy as np
import ml_dtypes

import concourse.bacc as bacc
import concourse.mybir as mybir
import concourse.tile as tile
from concourse.bass_utils import run_bass_kernel_spmd

P = 128
D = 256
B = 4096
N2 = 2 * B            # 8192 rows
NCORES = 8
NCH = 8               # 1024-row/col chunks
CW = 1024             # chunk width
TPC = 8               # [128, D] row-tiles per natural chunk
G = 4                 # psum column groups of 2048
M_TILES = 8
TEMP = 0.5
CSTAB = 2.0
SCALE_EXP = 2.0 / 256.0   # psum holds 256*s
LN16 = math.log(16.0)

f32 = mybir.dt.float32
bf16 = mybir.dt.bfloat16
fp8 = mybir.dt.float8e4
AF = mybir.ActivationFunctionType
OP = mybir.AluOpType
DR = mybir.MatmulPerfMode.DoubleRow


def _emit(tc, znat_d, zt_d, s_out, rawp_out, cs_outs):
    nc = tc.nc

    with tc.tile_pool(name="nat", bufs=NCH) as nat_pool, \
            tc.tile_pool(name="zth", bufs=2 * NCH) as zth_pool, \
            tc.tile_pool(name="zn8", bufs=NCH) as zn8_pool, \
            tc.tile_pool(name="invb", bufs=NCH) as invb_pool, \
            tc.tile_pool(name="sqt", bufs=2) as sq_pool, \
            tc.tile_pool(name="ett", bufs=2) as et_pool, \
            tc.tile_pool(name="etk", bufs=6) as etk_pool, \
            tc.tile_pool(name="small", bufs=4 * NCH + 8) as small_pool, \
            tc.tile_pool(name="acc", bufs=4) as acc_pool, \
            tc.tile_pool(name="dram", bufs=2, space="DRAM") as dram_pool, \
            tc.tile_pool(name="psum", bufs=2, space="PSUM") as psum_pool:

        # Pre-load the combined ln+exp activation table set (id 6:
        # natural_log_exp_and_others) so per-chunk Ln and the main-loop
        # Exp never thrash tables (the auto pass alternates sets 5/0).
        nc.scalar.add_instruction(
            mybir.InstLoadActFuncSet(name="preload_lnexp",
                                     act_func_set_id=6, ins=[], outs=[]))
        bias_t = acc_pool.tile([P, 1], f32, tag="bias", name="bias_t")
        nc.vector.memset(bias_t[:], -CSTAB)
        ln16_t = acc_pool.tile([P, 1], f32, tag="ln16", name="ln16_t")
        nc.vector.memset(ln16_t[:], LN16)
        rs = acc_pool.tile([P, M_TILES * 3], f32, tag="rs", name="rs")
        s_sb = acc_pool.tile([P, M_TILES], f32, tag="ssb", name="s_sb")
        rawp = acc_pool.tile([P, TPC], f32, tag="rawp", name="rawp")

        inv_d = dram_pool.tile([N2], bf16, tag="invd", name="inv_d")

        nats, zths, zn8s, invbs = [], [], [], []
        for c in range(NCH):
            # natural chunk: host pre-arranged to SBUF layout [128, 2048]
            nat = nat_pool.tile([P, TPC * D], fp8, tag="nat", name=f"nat{c}")
            nc.sync.dma_start(out=nat[:],
                              in_=znat_d[:, 2048 * c:2048 * (c + 1)])
            nats.append(nat)
            # transposed chunk halves: [128 d, 1024 cols] each
            hh = []
            for h in range(2):
                zth = zth_pool.tile([P, CW], fp8, tag="zth",
                                    name=f"zth{c}_{h}")
                nc.gpsimd.dma_start(out=zth[:],
                                    in_=zt_d[P * h:P * (h + 1),
                                             CW * c:CW * (c + 1)])
                hh.append(zth)
            zths.append(hh)

        # per-chunk pipeline: ss -> inv16 -> DRAM -> invb -> normalize,
        # so the main loop can start after chunks 0,1 (~10us).
        for c in range(NCH):
            sq = sq_pool.tile([P, TPC * D], bf16, tag="sq", name="sq")
            nc.vector.tensor_tensor(sq[:], nats[c][:], nats[c][:], op=OP.mult)
            ss_c = small_pool.tile([P, TPC], f32, tag="ss", name=f"ss{c}")
            nc.vector.reduce_sum(
                out=ss_c[:].unsqueeze(-1),
                in_=sq[:].rearrange("p (t d) -> p t d", t=TPC),
                axis=mybir.AxisListType.X)
            # inv16 = exp(-0.5 ln ss + ln 16)   (single preloaded table set)
            ln_c = small_pool.tile([P, TPC], f32, tag="ln", name=f"ln{c}")
            nc.scalar.activation(ln_c[:], ss_c[:], AF.Ln)
            iv_c = small_pool.tile([P, TPC], bf16, tag="iv", name=f"iv{c}")
            nc.scalar.activation(iv_c[:], ln_c[:], AF.Exp,
                                 scale=-0.5, bias=ln16_t[:])
            # DRAM roundtrip: linear write (addr = 1024c + 8p + t)
            nc.scalar.dma_start(
                out=inv_d[CW * c:CW * (c + 1)].rearrange("(p t) -> p t",
                                                         t=TPC),
                in_=iv_c[:])
            invb = invb_pool.tile([P, CW], bf16, tag="invb", name=f"invb{c}")
            nc.scalar.dma_start(
                out=invb[:],
                in_=inv_d[CW * c:CW * (c + 1)].unsqueeze(0)
                    .broadcast_to([P, CW]))
            invbs.append(invb)
            # normalize transposed chunk -> fp8 (x16 via inv16)
            zn8 = zn8_pool.tile([P, 2 * CW], fp8, tag="zn8", name=f"zn8{c}")
            for h in range(2):
                nc.vector.tensor_tensor(
                    zn8[:, CW * h:CW * (h + 1)], zths[c][h][:], invb[:],
                    op=OP.mult)
            zn8s.append(zn8)

        # raw pair dots: rolled rows 0..1023 (chunk 0) x 4096..5119 (chunk 4)
        tt = sq_pool.tile([P, TPC * D], bf16, tag="sq", name="tt")
        nc.vector.tensor_tensor(tt[:], nats[0][:], nats[4][:], op=OP.mult)
        nc.vector.reduce_sum(
            out=rawp[:].unsqueeze(-1),
            in_=tt[:].rearrange("p (t d) -> p t d", t=TPC),
            axis=mybir.AxisListType.X)
        nc.sync.dma_start(out=rawp_out, in_=rawp[:])

        # main loop: 1024 own rows x 8192 cols, fp8 DoubleRow (K=256/instr)
        def lhsT(m):
            # own rows: chunk-0 cols i = 8q + m  ->  [128, 2, 128]
            return zn8s[0][:].rearrange("p (h q t) -> p h t q", h=2,
                                        t=TPC)[:, :, m, :]

        def rhs(chunk, off):
            return zn8s[chunk][:].rearrange(
                "p (h w) -> p h w", h=2)[:, :, off:off + 512]

        # symmetric: rows 0..1023 x cols 0..5119 (i-chunks 0..4).
        # Column contributions for chunks 5..7 come from other cores via
        # colsums of chunks 1..3 (s_rc = s_cr); host adds them at combine.
        cs128 = {}
        for j in (1, 2, 3):
            t = acc_pool.tile([P, CW], f32, tag=f"cs{j}", name=f"cs128_{j}")
            nc.vector.memset(t[:], 0.0)
            cs128[j] = t

        def accum_cs(j, et_t, off):
            nc.gpsimd.dma_start(out=cs128[j][:], in_=et_t[:, off:off + CW],
                                accum_op=OP.add)

        for g in range(3):
            w = 2048 if g < 2 else 1024
            for m in range(M_TILES):
                ps = psum_pool.tile([P, w], f32, tag="ps", name=f"ps{g}_{m}")
                for c4 in range(w // 512):
                    nc.tensor.matmul(
                        out=ps[:, 512 * c4:512 * (c4 + 1)],
                        lhsT=lhsT(m),
                        rhs=rhs(2 * g + c4 // 2, 512 * (c4 % 2)),
                        start=True, stop=True, perf_mode=DR)
                pool = etk_pool if g < 2 else et_pool
                et = pool.tile([P, w], bf16, tag="et", name=f"et{g}_{m}")
                idx = 3 * m + g
                nc.scalar.activation(et[:], ps[:], AF.Exp, bias=bias_t[:],
                                     scale=SCALE_EXP,
                                     accum_out=rs[:, idx:idx + 1])
                if g == 0:
                    accum_cs(1, et, CW)
                elif g == 1:
                    accum_cs(2, et, 0)
                    accum_cs(3, et, CW)

        for j in (1, 2, 3):
            nc.sync.dma_start(out=cs_outs[j - 1], in_=cs128[j][:])
        nc.vector.reduce_sum(
            out=s_sb[:].unsqueeze(-1),
            in_=rs[:].rearrange("p (m g) -> p m g", g=3),
            axis=mybir.AxisListType.X)
        nc.sync.dma_start(out=s_out, in_=s_sb[:])


def build():
    nc = bacc.Bacc("TRN2", target_bir_lowering=False, debug=False)
    znat = nc.dram_tensor("znat", [P, N2 * D // P], fp8,
                          kind="ExternalInput").ap()
    zt = nc.dram_tensor("zt", [D, N2], fp8, kind="ExternalInput").ap()
    s_out = nc.dram_tensor("s_out", [P, M_TILES], f32,
                           kind="ExternalOutput").ap()
    rawp_out = nc.dram_tensor("rawp_out", [P, TPC], f32,
                              kind="ExternalOutput").ap()
    cs_outs = [nc.dram_tensor(f"cs{j}", [P, CW], f32,
                              kind="ExternalOutput").ap() for j in (1, 2, 3)]
    with tile.TileContext(nc) as tc:
        _emit(tc, znat, zt, s_out, rawp_out, cs_outs)
    nc.compile()
    return nc


_COLMAP = None


def _colmap():
    global _COLMAP
    if _COLMAP is None:
        i = np.arange(N2)
        _COLMAP = (i // CW) * CW + (i % TPC) * P + (i % CW) // TPC
    return _COLMAP


def make_in_maps(z_i, z_j):
    z_full = np.concatenate(
        [np.asarray(z_i, dtype=np.float32), np.asarray(z_j, dtype=np.float32)],
        axis=0)
    rmap = _colmap()
    maps = []
    for k in range(NCORES):
        zr = np.roll(z_full, -CW * k, axis=0)
        # SBUF layout: znat[p, (c n d)] = zr[1024c + 128n + p, d]
        znat = np.ascontiguousarray(
            zr.reshape(NCH, TPC, P, D).transpose(2, 0, 1, 3)
            .reshape(P, NCH * TPC * D)).astype(ml_dtypes.float8_e4m3fn)
        zt = np.ascontiguousarray(zr.T[:, rmap]).astype(ml_dtypes.float8_e4m3fn)
        maps.append({"znat": znat, "zt": zt})
    return maps


def combine(results, z_full):
    n = np.linalg.norm(z_full.astype(np.float64), axis=1)
    S = np.empty(N2, np.float64)
    pv = np.empty(N2, np.float64)
    pp = np.arange(P)[:, None]
    mm = np.arange(M_TILES)[None, :]
    for k in range(NCORES):
        gidx = ((CW * k + P * mm + pp) % N2).ravel()
        S[gidx] = results[k]["s_out"].astype(np.float64).ravel()
        pv[gidx] = results[k]["rawp_out"].astype(np.float64).ravel()
    il = np.arange(CW)
    qq, tt = il // TPC, il % TPC
    for k in range(NCORES):
        for j in (1, 2, 3):
            cs = results[k][f"cs{j}"].astype(np.float64).sum(axis=0)
            gr = (CW * (k + j) + P * tt + qq) % N2
            S[gr] += cs
    St = S - 1.0                       # drop diag (exp(2*d-2) ~= 1)
    lse = np.log(St) + CSTAB
    partner = (np.arange(N2) + B) % N2
    p = pv / (n * n[partner])
    loss = np.mean(lse - 2.0 * p)
    return np.asarray(loss, dtype=np.float32)


_NC_CACHE = None


def kernel(z_i, z_j):
    global _NC_CACHE
    if _NC_CACHE is None:
        _NC_CACHE = build()
    z_full = np.concatenate(
        [np.asarray(z_i, dtype=np.float32), np.asarray(z_j, dtype=np.float32)],
        axis=0)
    res = run_bass_kernel_spmd(
        _NC_CACHE, make_in_maps(z_i, z_j), list(range(NCORES))).results
    return combine(res, z_full)


# revision 17
# speedup vs baseline: 1.0565x; 1.0011x over previous
"""NT-Xent (SimCLR) contrastive loss on 8 Trainium2 NeuronCores — v2.

Math: z = concat(z_i, z_j) [2B, D], zn = z / ||z||_row,
logits = zn @ zn.T / T (diag masked), targets pair row r with r±B.
loss = mean_r(LSE_r - 2*p_r),  LSE_r = log(S_r - diag_r) + 2,
  S_r = sum_c exp(2*s_rc - 2),  p_r = zn_r . zn_partner(r).

Sharding: data-parallel rows. Core k gets z pre-rotated by 1024k rows
(host np.roll), computes its 1024 rows x all 8192 cols.

v2 pipeline (per core):
  Host feeds bf16 natural z AND a bf16 transposed copy whose columns are
  permuted per-1024-chunk as i = 8q + t (q=0..127, t=0..7, row=128t+q).
  That makes the device-computed inv vector land LINEARLY in DRAM so it
  can be broadcast-read back across partitions with unit-stride DMA.
  Per chunk c: ss (DVE square+reduce) -> inv16 = exp(-ln(ss)/2 + ln16)
  (ACT, shares the ln/exp table set with the main loop) -> DRAM write +
  partition-replicated read -> normalize transposed chunk to fp8 e4m3
  (x16 scale) -> fp8 DoubleRow matmuls (full K=256 per instr, 0.5
  cyc/col) -> ACT exp(psum*2/256 - 2) with accum_out row sums.
  Pair logits from bf16 raw dots (rows 0..1023 x 4096..5119); host
  divides by fp64 norms. Diagonal approximated as exp(0)=1 at combine
  (error ~1e-4 of the row sum).
Host combine = the all-reduce: assemble S, subtract diag, log, mean.
"""

import math

import numpy as np
import ml_dtypes

import concourse.bacc as bacc
import concourse.mybir as mybir
import concourse.tile as tile
from concourse.bass_utils import run_bass_kernel_spmd

P = 128
D = 256
B = 4096
N2 = 2 * B            # 8192 rows
NCORES = 8
NCH = 8               # 1024-row/col chunks
CW = 1024             # chunk width
TPC = 8               # [128, D] row-tiles per natural chunk
G = 4                 # psum column groups of 2048
M_TILES = 8
TEMP = 0.5
CSTAB = 2.0
SCALE_EXP = 2.0 / 256.0   # psum holds 256*s
LN16 = math.log(16.0)

f32 = mybir.dt.float32
bf16 = mybir.dt.bfloat16
fp8 = mybir.dt.float8e4
AF = mybir.ActivationFunctionType
OP = mybir.AluOpType
DR = mybir.MatmulPerfMode.DoubleRow


def _emit(tc, znat_d, zt_d, s_out, rawp_out, cs_outs):
    nc = tc.nc

    with tc.tile_pool(name="nat", bufs=NCH) as nat_pool, \
            tc.tile_pool(name="zth", bufs=2 * NCH) as zth_pool, \
            tc.tile_pool(name="zn8", bufs=NCH) as zn8_pool, \
            tc.tile_pool(name="invb", bufs=NCH) as invb_pool, \
            tc.tile_pool(name="sqt", bufs=2) as sq_pool, \
            tc.tile_pool(name="ett", bufs=2) as et_pool, \
            tc.tile_pool(name="etk", bufs=10) as etk_pool, \
            tc.tile_pool(name="small", bufs=4 * NCH + 8) as small_pool, \
            tc.tile_pool(name="acc", bufs=4) as acc_pool, \
            tc.tile_pool(name="dram", bufs=2, space="DRAM") as dram_pool, \
            tc.tile_pool(name="psum", bufs=2, space="PSUM") as psum_pool:

        # Pre-load the combined ln+exp activation table set (id 6:
        # natural_log_exp_and_others) so per-chunk Ln and the main-loop
        # Exp never thrash tables (the auto pass alternates sets 5/0).
        nc.scalar.add_instruction(
            mybir.InstLoadActFuncSet(name="preload_lnexp",
                                     act_func_set_id=6, ins=[], outs=[]))
        bias_t = acc_pool.tile([P, 1], f32, tag="bias", name="bias_t")
        nc.vector.memset(bias_t[:], -CSTAB)
        ln16_t = acc_pool.tile([P, 1], f32, tag="ln16", name="ln16_t")
        nc.vector.memset(ln16_t[:], LN16)
        rs = acc_pool.tile([P, M_TILES * 3], f32, tag="rs", name="rs")
        s_sb = acc_pool.tile([P, M_TILES], f32, tag="ssb", name="s_sb")
        rawp = acc_pool.tile([P, TPC], f32, tag="rawp", name="rawp")

        inv_d = dram_pool.tile([N2], bf16, tag="invd", name="inv_d")

        nats, zths, zn8s, invbs = [], [], [], []
        for c in range(NCH):
            # natural chunk: host pre-arranged to SBUF layout [128, 2048]
            nat = nat_pool.tile([P, TPC * D], fp8, tag="nat", name=f"nat{c}")
            nc.sync.dma_start(out=nat[:],
                              in_=znat_d[:, 2048 * c:2048 * (c + 1)])
            nats.append(nat)
            # transposed chunk halves: [128 d, 1024 cols] each
            hh = []
            for h in range(2):
                zth = zth_pool.tile([P, CW], fp8, tag="zth",
                                    name=f"zth{c}_{h}")
                nc.gpsimd.dma_start(out=zth[:],
                                    in_=zt_d[P * h:P * (h + 1),
                                             CW * c:CW * (c + 1)])
                hh.append(zth)
            zths.append(hh)

        # per-chunk pipeline: ss -> inv16 -> DRAM -> invb -> normalize,
        # so the main loop can start after chunks 0,1 (~10us).
        for c in range(NCH):
            sq = sq_pool.tile([P, TPC * D], bf16, tag="sq", name="sq")
            nc.vector.tensor_tensor(sq[:], nats[c][:], nats[c][:], op=OP.mult)
            ss_c = small_pool.tile([P, TPC], f32, tag="ss", name=f"ss{c}")
            nc.vector.reduce_sum(
                out=ss_c[:].unsqueeze(-1),
                in_=sq[:].rearrange("p (t d) -> p t d", t=TPC),
                axis=mybir.AxisListType.X)
            # inv16 = exp(-0.5 ln ss + ln 16)   (single preloaded table set)
            ln_c = small_pool.tile([P, TPC], f32, tag="ln", name=f"ln{c}")
            nc.scalar.activation(ln_c[:], ss_c[:], AF.Ln)
            iv_c = small_pool.tile([P, TPC], bf16, tag="iv", name=f"iv{c}")
            nc.scalar.activation(iv_c[:], ln_c[:], AF.Exp,
                                 scale=-0.5, bias=ln16_t[:])
            # DRAM roundtrip: linear write (addr = 1024c + 8p + t)
            nc.scalar.dma_start(
                out=inv_d[CW * c:CW * (c + 1)].rearrange("(p t) -> p t",
                                                         t=TPC),
                in_=iv_c[:])
            invb = invb_pool.tile([P, CW], bf16, tag="invb", name=f"invb{c}")
            nc.scalar.dma_start(
                out=invb[:],
                in_=inv_d[CW * c:CW * (c + 1)].unsqueeze(0)
                    .broadcast_to([P, CW]))
            invbs.append(invb)
            # normalize transposed chunk -> fp8 (x16 via inv16)
            zn8 = zn8_pool.tile([P, 2 * CW], fp8, tag="zn8", name=f"zn8{c}")
            for h in range(2):
                nc.vector.tensor_tensor(
                    zn8[:, CW * h:CW * (h + 1)], zths[c][h][:], invb[:],
                    op=OP.mult)
            zn8s.append(zn8)

        # raw pair dots: rolled rows 0..1023 (chunk 0) x 4096..5119 (chunk 4)
        tt = sq_pool.tile([P, TPC * D], bf16, tag="sq", name="tt")
        nc.vector.tensor_tensor(tt[:], nats[0][:], nats[4][:], op=OP.mult)
        nc.vector.reduce_sum(
            out=rawp[:].unsqueeze(-1),
            in_=tt[:].rearrange("p (t d) -> p t d", t=TPC),
            axis=mybir.AxisListType.X)
        nc.sync.dma_start(out=rawp_out, in_=rawp[:])

        # main loop: 1024 own rows x 8192 cols, fp8 DoubleRow (K=256/instr)
        def lhsT(m):
            # own rows: chunk-0 cols i = 8q + m  ->  [128, 2, 128]
            return zn8s[0][:].rearrange("p (h q t) -> p h t q", h=2,
                                        t=TPC)[:, :, m, :]

        def rhs(chunk, off):
            return zn8s[chunk][:].rearrange(
                "p (h w) -> p h w", h=2)[:, :, off:off + 512]

        # symmetric: rows 0..1023 x cols 0..5119 (i-chunks 0..4).
        # Column contributions for chunks 5..7 come from other cores via
        # colsums of chunks 1..3 (s_rc = s_cr); host adds them at combine.
        cs128 = {}
        for j in (1, 2, 3):
            t = acc_pool.tile([P, CW], f32, tag=f"cs{j}", name=f"cs128_{j}")
            nc.vector.memset(t[:], 0.0)
            cs128[j] = t

        def accum_cs(j, et_t, off):
            nc.gpsimd.dma_start(out=cs128[j][:], in_=et_t[:, off:off + CW],
                                accum_op=OP.add)

        for g in range(3):
            w = 2048 if g < 2 else 1024
            for m in range(M_TILES):
                ps = psum_pool.tile([P, w], f32, tag="ps", name=f"ps{g}_{m}")
                for c4 in range(w // 512):
                    nc.tensor.matmul(
                        out=ps[:, 512 * c4:512 * (c4 + 1)],
                        lhsT=lhsT(m),
                        rhs=rhs(2 * g + c4 // 2, 512 * (c4 % 2)),
                        start=True, stop=True, perf_mode=DR)
                pool = etk_pool if g < 2 else et_pool
                et = pool.tile([P, w], fp8 if g < 2 else bf16, tag="et",
                               name=f"et{g}_{m}")
                idx = 3 * m + g
                nc.scalar.activation(et[:], ps[:], AF.Exp, bias=bias_t[:],
                                     scale=SCALE_EXP,
                                     accum_out=rs[:, idx:idx + 1])
                if g == 0:
                    accum_cs(1, et, CW)
                elif g == 1:
                    accum_cs(2, et, 0)
                    accum_cs(3, et, CW)

        for j in (1, 2, 3):
            nc.sync.dma_start(out=cs_outs[j - 1], in_=cs128[j][:])
        nc.vector.reduce_sum(
            out=s_sb[:].unsqueeze(-1),
            in_=rs[:].rearrange("p (m g) -> p m g", g=3),
            axis=mybir.AxisListType.X)
        nc.sync.dma_start(out=s_out, in_=s_sb[:])


def build():
    nc = bacc.Bacc("TRN2", target_bir_lowering=False, debug=False)
    znat = nc.dram_tensor("znat", [P, N2 * D // P], fp8,
                          kind="ExternalInput").ap()
    zt = nc.dram_tensor("zt", [D, N2], fp8, kind="ExternalInput").ap()
    s_out = nc.dram_tensor("s_out", [P, M_TILES], f32,
                           kind="ExternalOutput").ap()
    rawp_out = nc.dram_tensor("rawp_out", [P, TPC], f32,
                              kind="ExternalOutput").ap()
    cs_outs = [nc.dram_tensor(f"cs{j}", [P, CW], f32,
                              kind="ExternalOutput").ap() for j in (1, 2, 3)]
    with tile.TileContext(nc) as tc:
        _emit(tc, znat, zt, s_out, rawp_out, cs_outs)
    nc.compile()
    return nc


_COLMAP = None


def _colmap():
    global _COLMAP
    if _COLMAP is None:
        i = np.arange(N2)
        _COLMAP = (i // CW) * CW + (i % TPC) * P + (i % CW) // TPC
    return _COLMAP


def make_in_maps(z_i, z_j):
    z_full = np.concatenate(
        [np.asarray(z_i, dtype=np.float32), np.asarray(z_j, dtype=np.float32)],
        axis=0)
    rmap = _colmap()
    maps = []
    for k in range(NCORES):
        zr = np.roll(z_full, -CW * k, axis=0)
        # SBUF layout: znat[p, (c n d)] = zr[1024c + 128n + p, d]
        znat = np.ascontiguousarray(
            zr.reshape(NCH, TPC, P, D).transpose(2, 0, 1, 3)
            .reshape(P, NCH * TPC * D)).astype(ml_dtypes.float8_e4m3fn)
        zt = np.ascontiguousarray(zr.T[:, rmap]).astype(ml_dtypes.float8_e4m3fn)
        maps.append({"znat": znat, "zt": zt})
    return maps


def combine(results, z_full):
    n = np.linalg.norm(z_full.astype(np.float64), axis=1)
    S = np.empty(N2, np.float64)
    pv = np.empty(N2, np.float64)
    pp = np.arange(P)[:, None]
    mm = np.arange(M_TILES)[None, :]
    for k in range(NCORES):
        gidx = ((CW * k + P * mm + pp) % N2).ravel()
        S[gidx] = results[k]["s_out"].astype(np.float64).ravel()
        pv[gidx] = results[k]["rawp_out"].astype(np.float64).ravel()
    il = np.arange(CW)
    qq, tt = il // TPC, il % TPC
    for k in range(NCORES):
        for j in (1, 2, 3):
            cs = results[k][f"cs{j}"].astype(np.float64).sum(axis=0)
            gr = (CW * (k + j) + P * tt + qq) % N2
            S[gr] += cs
    St = S - 1.0                       # drop diag (exp(2*d-2) ~= 1)
    lse = np.log(St) + CSTAB
    partner = (np.arange(N2) + B) % N2
    p = pv / (n * n[partner])
    loss = np.mean(lse - 2.0 * p)
    return np.asarray(loss, dtype=np.float32)


_NC_CACHE = None


def kernel(z_i, z_j):
    global _NC_CACHE
    if _NC_CACHE is None:
        _NC_CACHE = build()
    z_full = np.concatenate(
        [np.asarray(z_i, dtype=np.float32), np.asarray(z_j, dtype=np.float32)],
        axis=0)
    res = run_bass_kernel_spmd(
        _NC_CACHE, make_in_maps(z_i, z_j), list(range(NCORES))).results
    return combine(res, z_full)
